# revision 11
# baseline (speedup 1.0000x reference)
import sys, os
sys.path.insert(0, '/opt/trn_rl_repo')
import numpy as np
import concourse.bass as bass
import concourse.tile as tile
from concourse import bacc, mybir
from concourse.bass_utils import run_bass_kernel_spmd
from concourse.masks import make_identity

dt = mybir.dt
f32, f32r, bf16 = dt.float32, dt.float32r, dt.bfloat16

B, S, D, DFF, H, V, L = 2, 1024, 1024, 4096, 16, 32000, 8
DH = D // H          # 64
NC = 8               # cores
TOK = (B * S) // NC  # 256 tokens per core
NTB = TOK // 128     # 2 token blocks per core
NDC = D // 128       # 8 d-chunks
NKC = (S) // 128     # 8 key chunks per batch
NFFC = DFF // 128    # 32
VSH = V // NC        # 4000 vocab per core
VPAD = 4096          # padded vocab slice
EPS = 1e-5
HKV = TOK * D        # elements per kv AllGather half (K half + V half)


def _pe():
    pos = np.arange(S, dtype=np.float32)[:, None]
    div = np.exp(np.arange(0, D, 2, dtype=np.float32) * (-np.log(10000.0) / D))
    pe = np.zeros((S, D), dtype=np.float32)
    pe[:, 0::2] = np.sin(pos * div)
    pe[:, 1::2] = np.cos(pos * div)
    return pe


def build_program(ln1_triv, ln2_triv, b2_triv, lnf_triv, pb_triv):
    nc = bacc.Bacc("TRN2", target_bir_lowering=False, debug=False,
                   enable_asserts=False, num_devices=NC)

    # ---- DRAM inputs (shared weights) ----
    wq_d = nc.dram_tensor("wq", [L, D, D], bf16, kind="ExternalInput").ap()
    wk_d = nc.dram_tensor("wk", [L, D, D], bf16, kind="ExternalInput").ap()
    wv_d = nc.dram_tensor("wv", [L, D, D], bf16, kind="ExternalInput").ap()
    wo_d = nc.dram_tensor("wo", [L, D, D], bf16, kind="ExternalInput").ap()
    w1_d = nc.dram_tensor("w1", [L, D, DFF], bf16, kind="ExternalInput").ap()
    w2_d = nc.dram_tensor("w2", [L, DFF, D], bf16, kind="ExternalInput").ap()
    b1_d = nc.dram_tensor("b1", [L, DFF], f32, kind="ExternalInput").ap()
    # ---- per-core inputs ----
    embx_d = nc.dram_tensor("embx", [TOK, D], f32, kind="ExternalInput").ap()
    pe_d = nc.dram_tensor("pe", [TOK, D], f32, kind="ExternalInput").ap()
    mt_d = nc.dram_tensor("mt", [NKC, 128, TOK], bf16, kind="ExternalInput").ap()
    pw_d = nc.dram_tensor("pw", [D, VPAD], bf16, kind="ExternalInput").ap()
    if not pb_triv:
        pbb_d = nc.dram_tensor("pbb", [128, VPAD], f32, kind="ExternalInput").ap()
    # optional non-trivial affine params (pre-broadcast on host)
    if not (ln1_triv and ln2_triv):
        lngb_d = nc.dram_tensor("lngb", [L, 4, 128, D], f32, kind="ExternalInput").ap()
    if not b2_triv:
        b2b_d = nc.dram_tensor("b2b", [L, 128, D], f32, kind="ExternalInput").ap()
    if not lnf_triv:
        fgb_d = nc.dram_tensor("fgb", [2, 128, D], f32, kind="ExternalInput").ap()
    # ---- output: token-major logits ----
    out_d = nc.dram_tensor("logits", [B * S, VPAD], f32, kind="ExternalOutput").ap()

    from contextlib import ExitStack
    with tile.TileContext(nc) as tc:
        with ExitStack() as ctx0:
            # pools that live for the whole program
            cpool = ctx0.enter_context(tc.tile_pool(name="const", bufs=1))
            dpool = ctx0.enter_context(tc.tile_pool(name="dram", bufs=1, space="DRAM"))

            # ---------------- constants ----------------
            ident = cpool.tile([128, 128], bf16, tag="ident")
            make_identity(nc, ident[:])
            ones = cpool.tile([128, 64], bf16, tag="ones")
            nc.vector.memset(ones[:], 1.0)
            epst = cpool.tile([128, 1], f32, tag="eps")
            nc.vector.memset(epst[:], EPS)
            mtt = cpool.tile([128, NKC, TOK], bf16, tag="mt")
            nc.sync.dma_start(out=mtt[:], in_=mt_d.rearrange("kc p q -> p kc q"))
            b1t = cpool.tile([128, L, NFFC], f32, tag="b1")
            nc.sync.dma_start(out=b1t[:], in_=b1_d.rearrange("l (fc p) -> p l fc", p=128))
            if not (ln1_triv and ln2_triv):
                lngb = cpool.tile([128, L, 4, D], f32, tag="lngb")
                nc.sync.dma_start(out=lngb[:], in_=lngb_d.rearrange("l f p d -> p l f d"))
            if not b2_triv:
                b2b = cpool.tile([128, L, D], f32, tag="b2b")
                nc.sync.dma_start(out=b2b[:], in_=b2b_d.rearrange("l p d -> p l d"))
            if not lnf_triv:
                fgb = cpool.tile([128, 2, D], f32, tag="fgb")
                nc.sync.dma_start(out=fgb[:], in_=fgb_d.rearrange("f p d -> p f d"))

            # collective DRAM buffers: kv AllGather in 2 head-halves
            kv_in = [dpool.tile([HKV], bf16, tag=f"kv_in{g}", name=f"kv_in{g}")
                     for g in range(2)]
            kv_out = [dpool.tile([4, HKV], bf16, tag=f"kv_out{g}", name=f"kv_out{g}")
                      for g in range(2)]
            ag2_in = dpool.tile([TOK * D], bf16, tag="ag2_in")
            ag2_out = dpool.tile([NC, TOK * D], bf16, tag="ag2_out", addr_space="Shared")

            with ExitStack() as ctx:
                rpool = ctx.enter_context(tc.tile_pool(name="resid", bufs=1))
                atpool = ctx.enter_context(tc.tile_pool(name="aT", bufs=2))
                qkvpool = ctx.enter_context(tc.tile_pool(name="qkv", bufs=1))
                vapool = ctx.enter_context(tc.tile_pool(name="vall", bufs=1))
                wpool = ctx.enter_context(tc.tile_pool(name="wch", bufs=4))
                w2pool = ctx.enter_context(tc.tile_pool(name="w2c", bufs=3))
                utpool = ctx.enter_context(tc.tile_pool(name="ut", bufs=32))
                htpool = ctx.enter_context(tc.tile_pool(name="ht", bufs=4))
                apool = ctx.enter_context(tc.tile_pool(name="att", bufs=8))
                alnpool = ctx.enter_context(tc.tile_pool(name="aln", bufs=2))
                spool = ctx.enter_context(tc.tile_pool(name="small", bufs=4))
                recpool = ctx.enter_context(tc.tile_pool(name="recp", bufs=2))
                psmm = ctx.enter_context(tc.tile_pool(name="ps_mm", bufs=4, space="PSUM"))
                pso = ctx.enter_context(tc.tile_pool(name="ps_o", bufs=4, space="PSUM"))

                # resident V (+ones col) for attention: [k, kc, head, dh|1]
                vall = vapool.tile([128, NKC, H, DH + 1], bf16, tag="vall", name="vall")
                nc.vector.memset(vall[:], 1.0)

                # ---------------- embedding ----------------
                h = [rpool.tile([128, D], f32, tag=f"h{tb}", name=f"h{tb}") for tb in range(NTB)]
                for tb in range(NTB):
                    et = alnpool.tile([128, D], f32, tag="aln")
                    pt = alnpool.tile([128, D], f32, tag="aln")
                    nc.sync.dma_start(out=et, in_=embx_d[tb * 128:(tb + 1) * 128, :])
                    nc.sync.dma_start(out=pt, in_=pe_d[tb * 128:(tb + 1) * 128, :])
                    nc.vector.tensor_scalar(out=et[:], in0=et[:], scalar1=float(np.sqrt(D)),
                                            scalar2=None, op0=mybir.AluOpType.mult)
                    nc.vector.tensor_add(h[tb][:], et[:], pt[:])

                def layer_norm(src_tiles, dst_tag, gb=None):
                    """LN along free dim of token-major tiles; returns new tiles."""
                    outs = []
                    for tb in range(NTB):
                        st = spool.tile([128, 2, 6], f32, tag="bnst")
                        xin = src_tiles[tb][:].rearrange("p (g d) -> p g d", g=2)
                        for g in range(2):
                            nc.vector.bn_stats(out=st[:, g, :], in_=xin[:, g, :])
                        mv = spool.tile([128, 2], f32, tag="bnmv")
                        nc.vector.bn_aggr(out=mv[:], in_=st[:])
                        std = spool.tile([128, 1], f32, tag="bnsd")
                        nc.scalar.activation(out=std[:], in_=mv[:, 1:2],
                                             func=mybir.ActivationFunctionType.Sqrt,
                                             bias=epst[:], scale=1.0)
                        nc.vector.reciprocal(out=std[:], in_=std[:])
                        at = alnpool.tile([128, D], bf16, tag=dst_tag)
                        nc.vector.tensor_scalar(out=at[:], in0=src_tiles[tb][:],
                                                scalar1=mv[:, 0:1], scalar2=std[:],
                                                op0=mybir.AluOpType.subtract,
                                                op1=mybir.AluOpType.mult)
                        if gb is not None:
                            gt, bt = gb
                            nc.vector.tensor_mul(at[:], at[:], gt)
                            nc.vector.tensor_add(at[:], at[:], bt)
                        outs.append(at)
                    return outs

                def transpose_to_aT(src_tiles):
                    """token-major [128, D] x NTB -> aT [128, NDC, TOK]."""
                    at = atpool.tile([128, NDC, TOK], bf16, tag="aT")
                    for tb in range(NTB):
                        for dc in range(NDC):
                            ps = psmm.tile([128, 128], bf16, tag="mm")
                            nc.tensor.transpose(ps[:], src_tiles[tb][:, dc * 128:(dc + 1) * 128], ident[:])
                            nc.vector.tensor_copy(out=at[:, dc, tb * 128:(tb + 1) * 128], in_=ps[:])
                    return at

                # ---------------- layers ----------------
                for li in range(L):
                    ln1_gb = None
                    if not ln1_triv:
                        ln1_gb = (lngb[:, li, 0, :], lngb[:, li, 1, :])
                    aln = layer_norm(h, "aln", ln1_gb)
                    aT = transpose_to_aT(aln)

                    # --- K/V projections + AllGather, split in two head-halves ---
                    kloc = qkvpool.tile([128, NDC, TOK], bf16, tag="kloc")
                    vloc = [qkvpool.tile([128, D], bf16, tag=f"vloc{tb}", name=f"vloc{tb}")
                            for tb in range(NTB)]
                    wk_src = wk_d[li].rearrange("(dc p) j -> p dc j", p=128)
                    wv_src = wv_d[li].rearrange("(dc p) j -> p dc j", p=128)
                    for g in range(2):
                        # K proj: head-half g => j-chunks 4g..4g+3
                        wc = wpool.tile([128, NDC, 512], bf16, tag="wch")
                        nc.sync.dma_start(out=wc, in_=wk_src[:, :, g * 512:(g + 1) * 512])
                        for oc in range(4):
                            jc = g * 4 + oc
                            ps = psmm.tile([128, TOK], f32, tag="mm")
                            for dc in range(NDC):
                                nc.tensor.matmul(ps[:], wc[:, dc, oc * 128:(oc + 1) * 128],
                                                 aT[:, dc, :], start=(dc == 0), stop=(dc == NDC - 1))
                            nc.vector.tensor_copy(out=kloc[:, jc, :], in_=ps[:])
                        # V proj: dims g*512..g*512+512 (token-major out)
                        wc = wpool.tile([128, NDC, 512], bf16, tag="wch")
                        nc.sync.dma_start(out=wc, in_=wv_src[:, :, g * 512:(g + 1) * 512])
                        for tb in range(NTB):
                            ps = psmm.tile([128, 512], f32, tag="mm")
                            for dc in range(NDC):
                                nc.tensor.matmul(ps[:], aT[:, dc, tb * 128:(tb + 1) * 128],
                                                 wc[:, dc, :], start=(dc == 0), stop=(dc == NDC - 1))
                            nc.vector.tensor_copy(out=vloc[tb][:, g * 512:(g + 1) * 512], in_=ps[:])
                        # pack + AllGather this half
                        nc.sync.dma_start(
                            out=kv_in[g][0:HKV // 2].rearrange("(jc p k) -> p jc k", p=128, k=TOK),
                            in_=kloc[:, g * 4:(g + 1) * 4, :])
                        for tb in range(NTB):
                            nc.sync.dma_start(
                                out=kv_in[g][HKV // 2:HKV].rearrange(
                                    "(tb p j) -> p tb j", p=128, j=512)[:, tb, :],
                                in_=vloc[tb][:, g * 512:(g + 1) * 512])
                        nc.gpsimd.collective_compute(
                            "AllGather", mybir.AluOpType.bypass,
                            replica_groups=[[0, 1, 2, 3], [4, 5, 6, 7]],
                            ins=[kv_in[g][:].opt()], outs=[kv_out[g][:].opt()])

                    # --- Q projection (overlaps the AllGathers) ---
                    qT = qkvpool.tile([128, NDC, TOK], bf16, tag="qT")
                    wsrc = wq_d[li].rearrange("(dc p) j -> p dc j", p=128)
                    for cg in range(2):
                        wc = wpool.tile([128, NDC, 512], bf16, tag="wch")
                        nc.sync.dma_start(out=wc, in_=wsrc[:, :, cg * 512:(cg + 1) * 512])
                        for oc in range(4):
                            jc = cg * 4 + oc
                            ps = psmm.tile([128, TOK], f32, tag="mm")
                            for dc in range(NDC):
                                nc.tensor.matmul(ps[:], wc[:, dc, oc * 128:(oc + 1) * 128],
                                                 aT[:, dc, :], start=(dc == 0), stop=(dc == NDC - 1))
                            nc.vector.tensor_copy(out=qT[:, jc, :], in_=ps[:])

                    # --- attention ---
                    oall = qkvpool.tile([128, NDC, TOK], bf16, tag="oall")
                    for g in range(2):
                        kv_k = kv_out[g][:, 0:HKV // 2].rearrange(
                            "s (jc p k) -> p s jc k", jc=4, p=128, k=TOK)
                        kv_v = kv_out[g][:, HKV // 2:HKV].rearrange(
                            "s (tb p j) -> p s tb j", tb=NTB, p=128, j=512)
                        # K tile for the first head-pair goes first: scores can
                        # start the moment the gather lands (AV needs vall later).
                        ktiles = [htpool.tile([128, 4, TOK], bf16, tag="kt", name=f"kt{g}{hl}")
                                  for hl in range(4)]
                        nc.scalar.dma_start(out=ktiles[0], in_=kv_k[:, :, 0, :])
                        # V (+ones) into resident vall for heads of this half
                        for s in range(4):
                            for tb in range(NTB):
                                nc.scalar.dma_start(
                                    out=vall[:, s * 2 + tb, g * 8:(g + 1) * 8, 0:DH],
                                    in_=kv_v[:, s, tb, :].rearrange("p (hh d) -> p hh d", d=DH))
                        for hl in range(1, 4):
                            nc.scalar.dma_start(out=ktiles[hl], in_=kv_k[:, :, hl, :])
                        for hl in range(4):
                            hp = g * 4 + hl
                            ktile = ktiles[hl]
                            ps_o = [pso.tile([128, TOK], f32, tag="o", name=f"pso{hh}")
                                    for hh in range(2)]
                            at2s = [None, None]
                            for kg in range(4):
                                prev = at2s
                                at2s = [None, None]
                                for hh in range(2):
                                    hb = hh * 64
                                    ps_s = psmm.tile([128, 512], f32, tag="mm")
                                    for k2 in range(2):
                                        kc = kg * 2 + k2
                                        nc.tensor.matmul(
                                            ps_s[:, k2 * 256:(k2 + 1) * 256],
                                            ktile[hb:hb + 64, kc // 2, (kc % 2) * 128:(kc % 2) * 128 + 128],
                                            qT[hb:hb + 64, hp, :], start=True, stop=True)
                                    et = apool.tile([128, 512], bf16, tag="att")
                                    nc.scalar.activation(out=et[:], in_=ps_s[:],
                                                         func=mybir.ActivationFunctionType.Exp,
                                                         scale=float(1.0 / np.sqrt(DH)))
                                    at2 = apool.tile([128, 512], bf16, tag="att")
                                    nc.gpsimd.tensor_mul(
                                        at2[:], et[:],
                                        mtt[:, kg * 2:(kg + 1) * 2, :].rearrange("p a b -> p (a b)"))
                                    at2s[hh] = at2
                                if kg > 0:
                                    for hh in range(2):
                                        for k2 in range(2):
                                            kc = (kg - 1) * 2 + k2
                                            nc.tensor.matmul(
                                                ps_o[hh][0:DH + 1, :],
                                                vall[:, kc, hp * 2 + hh, :],
                                                prev[hh][:, k2 * 256:(k2 + 1) * 256],
                                                start=(kc == 0), stop=False)
                            for hh in range(2):
                                for k2 in range(2):
                                    kc = 6 + k2
                                    nc.tensor.matmul(
                                        ps_o[hh][0:DH + 1, :],
                                        vall[:, kc, hp * 2 + hh, :],
                                        at2s[hh][:, k2 * 256:(k2 + 1) * 256],
                                        start=False, stop=(kc == NKC - 1))
                            for hh in range(2):
                                hb = hh * 64
                                rec = recpool.tile([128, TOK], bf16, tag="rec")
                                with nc.allow_low_precision(reason="softmax denom recip"):
                                    nc.vector.reciprocal(out=rec[DH:DH + 1, :],
                                                         in_=ps_o[hh][DH:DH + 1, :])
                                ps_rb = pso.tile([128, TOK], f32, tag="o", name=f"psrb{hh}")
                                nc.tensor.matmul(ps_rb[0:64, :], ones[DH:DH + 1, 0:64],
                                                 rec[DH:DH + 1, :], start=True, stop=True)
                                rb = apool.tile([128, TOK], f32, tag="attf")
                                nc.vector.tensor_copy(out=rb[0:64, :], in_=ps_rb[0:64, :])
                                nc.vector.tensor_mul(oall[hb:hb + 64, hp, :],
                                                     ps_o[hh][0:64, :], rb[0:64, :])

                    # --- Wo projection (token-major out) + residual ---
                    wsrc = wo_d[li].rearrange("(jc p) o -> p jc o", p=128)
                    for cg in range(2):
                        wc = wpool.tile([128, NDC, 512], bf16, tag="wch")
                        nc.sync.dma_start(out=wc, in_=wsrc[:, :, cg * 512:(cg + 1) * 512])
                        for tb in range(NTB):
                            ps = psmm.tile([128, 512], f32, tag="mm")
                            for jc in range(NDC):
                                nc.tensor.matmul(ps[:], oall[:, jc, tb * 128:(tb + 1) * 128],
                                                 wc[:, jc, :], start=(jc == 0), stop=(jc == NDC - 1))
                            nc.vector.tensor_add(h[tb][:, cg * 512:(cg + 1) * 512],
                                                 h[tb][:, cg * 512:(cg + 1) * 512], ps[:])

                    # --- FFN ---
                    ln2_gb = None
                    if not ln2_triv:
                        ln2_gb = (lngb[:, li, 2, :], lngb[:, li, 3, :])
                    f_ln = layer_norm(h, "aln", ln2_gb)
                    fT = transpose_to_aT(f_ln)

                    uts = []
                    wsrc = w1_d[li].rearrange("(dc p) j -> p dc j", p=128)
                    for cg in range(NFFC // 4):
                        wc = wpool.tile([128, NDC, 512], bf16, tag="wch")
                        nc.sync.dma_start(out=wc, in_=wsrc[:, :, cg * 512:(cg + 1) * 512])
                        for oc in range(4):
                            fc = cg * 4 + oc
                            ps = psmm.tile([128, TOK], f32, tag="mm")
                            for dc in range(NDC):
                                nc.tensor.matmul(ps[:], wc[:, dc, oc * 128:(oc + 1) * 128],
                                                 fT[:, dc, :], start=(dc == 0), stop=(dc == NDC - 1))
                            ut = utpool.tile([128, TOK], bf16, tag="ut")
                            nc.vector.tensor_scalar(out=ut[:], in0=ps[:],
                                                    scalar1=b1t[:, li, fc:fc + 1], scalar2=0.0,
                                                    op0=mybir.AluOpType.add,
                                                    op1=mybir.AluOpType.max)
                            uts.append(ut)

                    # --- W2: 4 psum chains (tb x og), accumulate over ffc ---
                    wsrc = w2_d[li].rearrange("(fc p) o -> p fc o", p=128)
                    chains = {}
                    for tb in range(NTB):
                        for og in range(2):
                            chains[(tb, og)] = psmm.tile([128, 512], f32, tag="mm", name=f"w2ch{tb}{og}")
                    for fcg in range(NFFC // 2):
                        wc = w2pool.tile([128, 2, D], bf16, tag="w2c")
                        nc.sync.dma_start(out=wc, in_=wsrc[:, fcg * 2:fcg * 2 + 2, :])
                        for f2 in range(2):
                            fc = fcg * 2 + f2
                            for tb in range(NTB):
                                for og in range(2):
                                    nc.tensor.matmul(chains[(tb, og)][:],
                                                     uts[fc][:, tb * 128:(tb + 1) * 128],
                                                     wc[:, f2, og * 512:(og + 1) * 512],
                                                     start=(fc == 0), stop=(fc == NFFC - 1))
                    for tb in range(NTB):
                        for og in range(2):
                            nc.vector.tensor_add(h[tb][:, og * 512:(og + 1) * 512],
                                                 h[tb][:, og * 512:(og + 1) * 512],
                                                 chains[(tb, og)][:])
                        if not b2_triv:
                            nc.vector.tensor_add(h[tb][:], h[tb][:], b2b[:, li, :])

                # ---------------- final LN + AllGather ----------------
                fin_gb = None if lnf_triv else (fgb[:, 0, :], fgb[:, 1, :])
                fin = layer_norm(h, "aln", fin_gb)
                finT = transpose_to_aT(fin)
                nc.sync.dma_start(
                    out=ag2_in[:].rearrange("(jc p t) -> p jc t", p=128, t=TOK),
                    in_=finT[:])
                nc.gpsimd.collective_compute(
                    "AllGather", mybir.AluOpType.bypass,
                    replica_groups=[[0, 1, 2, 3, 4, 5, 6, 7]],
                    ins=[ag2_in[:].opt()], outs=[ag2_out[:].opt()])

            # ---------------- vocab projection (fresh pools) ----------------
            with ExitStack() as ctx2:
                ppool = ctx2.enter_context(tc.tile_pool(name="pw", bufs=2))
                hpool2 = ctx2.enter_context(tc.tile_pool(name="hts", bufs=3))
                epool = ctx2.enter_context(tc.tile_pool(name="evac", bufs=4))
                pchain = ctx2.enter_context(tc.tile_pool(name="ps_p", bufs=4, space="PSUM"))

                if not pb_triv:
                    pbbt = cpool.tile([128, VPAD], f32, tag="pbb")
                    nc.sync.dma_start(out=pbbt[:], in_=pbb_d)

                hfull = ag2_out[:].rearrange("s (jc p t) -> p s jc t", jc=NDC, p=128, t=TOK)
                pwsrc = pw_d.rearrange("(dc p) v -> p dc v", p=128)
                for vh in range(2):
                    pwall = ppool.tile([128, NDC, 2048], bf16, tag="pwall")
                    nc.sync.dma_start(out=pwall, in_=pwsrc[:, :, vh * 2048:(vh + 1) * 2048])
                    for ts in range(16):
                        sr, tb2 = ts // 2, ts % 2
                        hts = hpool2.tile([128, NDC, 128], bf16, tag="hts")
                        nc.sync.dma_start(out=hts, in_=hfull[:, sr, :, tb2 * 128:(tb2 + 1) * 128])
                        for vq in range(4):
                            ps = pchain.tile([128, 512], f32, tag="pj")
                            for dc in range(NDC):
                                nc.tensor.matmul(ps[:], hts[:, dc, :],
                                                 pwall[:, dc, vq * 512:(vq + 1) * 512],
                                                 start=(dc == 0), stop=(dc == NDC - 1))
                            lsb = epool.tile([128, 512], f32, tag="ev")
                            if pb_triv:
                                nc.vector.tensor_copy(out=lsb[:], in_=ps[:])
                            else:
                                nc.vector.tensor_add(
                                    lsb[:], ps[:],
                                    pbbt[:, vh * 2048 + vq * 512: vh * 2048 + (vq + 1) * 512])
                            nc.sync.dma_start(
                                out=out_d[ts * 128:(ts + 1) * 128,
                                          vh * 2048 + vq * 512: vh * 2048 + (vq + 1) * 512],
                                in_=lsb[:])
    nc.compile()
    return nc


def kernel(**inputs):
    x = np.asarray(inputs["x"])
    mask = np.asarray(inputs["mask"])
    emb = np.asarray(inputs["emb"], dtype=np.float32)

    ln1_g = np.asarray(inputs["ln1_g"], dtype=np.float32)
    ln1_b = np.asarray(inputs["ln1_b"], dtype=np.float32)
    ln2_g = np.asarray(inputs["ln2_g"], dtype=np.float32)
    ln2_b = np.asarray(inputs["ln2_b"], dtype=np.float32)
    lnf_g = np.asarray(inputs["lnf_g"], dtype=np.float32)
    lnf_b = np.asarray(inputs["lnf_b"], dtype=np.float32)
    b2 = np.asarray(inputs["b2"], dtype=np.float32)
    pb_full = np.asarray(inputs["projb"], dtype=np.float32)

    ln1_triv = bool(np.all(ln1_g == 1) and np.all(ln1_b == 0))
    ln2_triv = bool(np.all(ln2_g == 1) and np.all(ln2_b == 0))
    lnf_triv = bool(np.all(lnf_g == 1) and np.all(lnf_b == 0))
    b2_triv = bool(np.all(b2 == 0))
    pb_triv = bool(np.all(pb_full == 0))

    nc = build_program(ln1_triv, ln2_triv, b2_triv, lnf_triv, pb_triv)

    pe_full = _pe()
    ids = np.asarray(x).reshape(B * S)
    m2d = np.asarray(mask[0, 0], dtype=np.float32)  # [S(q), S(k)]
    pw_full = np.asarray(inputs["projW"], dtype=np.float32)

    import ml_dtypes
    bfl = ml_dtypes.bfloat16
    shared = {
        "wq": np.ascontiguousarray(np.asarray(inputs["Wq"], dtype=bfl)),
        "wk": np.ascontiguousarray(np.asarray(inputs["Wk"], dtype=bfl)),
        "wv": np.ascontiguousarray(np.asarray(inputs["Wv"], dtype=bfl)),
        "wo": np.ascontiguousarray(np.asarray(inputs["Wo"], dtype=bfl)),
        "w1": np.ascontiguousarray(np.asarray(inputs["W1"], dtype=bfl)),
        "w2": np.ascontiguousarray(np.asarray(inputs["W2"], dtype=bfl)),
        "b1": np.ascontiguousarray(inputs["b1"], dtype=np.float32),
    }
    if not (ln1_triv and ln2_triv):
        lngb = np.stack([
            np.broadcast_to(ln1_g[:, None, :], (L, 128, D)),
            np.broadcast_to(ln1_b[:, None, :], (L, 128, D)),
            np.broadcast_to(ln2_g[:, None, :], (L, 128, D)),
            np.broadcast_to(ln2_b[:, None, :], (L, 128, D)),
        ], axis=1)
        shared["lngb"] = np.ascontiguousarray(lngb, dtype=np.float32)
    if not b2_triv:
        shared["b2b"] = np.ascontiguousarray(
            np.broadcast_to(b2[:, None, :], (L, 128, D)), dtype=np.float32)
    if not lnf_triv:
        shared["fgb"] = np.ascontiguousarray(
            np.stack([np.broadcast_to(lnf_g[None, :], (128, D)),
                      np.broadcast_to(lnf_b[None, :], (128, D))]), dtype=np.float32)

    in_maps = []
    for c in range(NC):
        b = c // 4
        q0 = (c % 4) * TOK
        sl = slice(b * S + q0, b * S + q0 + TOK)
        embx = np.ascontiguousarray(emb[ids[sl]], dtype=np.float32)
        pes = np.ascontiguousarray(pe_full[q0:q0 + TOK], dtype=np.float32)
        # mask tiles in scores-T layout: mt[kc, k, q] = mask[q0+q, kc*128+k]
        msl = m2d[q0:q0 + TOK, :]  # [TOK, S]
        mt = np.ascontiguousarray(
            np.asarray(msl.T.reshape(NKC, 128, TOK), dtype=bfl))
        pw = np.zeros((D, VPAD), dtype=bfl)
        pw[:, :VSH] = np.asarray(pw_full[:, c * VSH:(c + 1) * VSH], dtype=bfl)
        im = dict(shared)
        im.update({"embx": embx, "pe": pes, "mt": mt, "pw": pw})
        if not pb_triv:
            pbb = np.zeros((128, VPAD), dtype=np.float32)
            pbb[:, :VSH] = pb_full[None, c * VSH:(c + 1) * VSH]
            im["pbb"] = np.ascontiguousarray(pbb)
        in_maps.append(im)

    trace = bool(int(os.environ.get("KERNEL_TRACE", "0")))
    if trace:
        _install_trace_hook()
    res = run_bass_kernel_spmd(nc, in_maps, core_ids=list(range(NC)), trace=trace)
    if trace:
        kernel.last_exec_time_ns = res.exec_time_ns

    parts = [res.results[c]["logits"][:, :VSH] for c in range(NC)]
    full = np.concatenate(parts, axis=1)          # [B*S, V]
    return np.ascontiguousarray(full.reshape(B, S, V))


def _install_trace_hook():
    import types
    if 'antenv.axon_hooks' in sys.modules:
        return
    try:
        import trn_agent_boot.trn_boot as trn_boot
        mod = types.ModuleType('antenv.axon_hooks')
        _hook = [None]
        mod.set_axon_ntff_profile_hook = lambda hk: _hook.__setitem__(0, hk)
        mod.get_axon_ntff_profile_hook = lambda: _hook[0]
        sys.modules['antenv.axon_hooks'] = mod
        import antenv
        antenv.axon_hooks = mod
        mod.set_axon_ntff_profile_hook(
            trn_boot._ntff_profile_via_ctypes('/opt/axon/libaxon_pjrt.so'))
    except Exception as e:
        print(f"trace hook unavailable: {e}", file=sys.stderr)


# revision 16
# speedup vs baseline: 1.0127x; 1.0127x over previous
import sys, os
sys.path.insert(0, '/opt/trn_rl_repo')
import numpy as np
import concourse.bass as bass
import concourse.tile as tile
from concourse import bacc, mybir
from concourse.bass_utils import run_bass_kernel_spmd
from concourse.masks import make_identity

dt = mybir.dt
f32, f32r, bf16 = dt.float32, dt.float32r, dt.bfloat16

B, S, D, DFF, H, V, L = 2, 1024, 1024, 4096, 16, 32000, 8
DH = D // H          # 64
NC = 8               # cores
TOK = (B * S) // NC  # 256 tokens per core
NTB = TOK // 128     # 2 token blocks per core
NDC = D // 128       # 8 d-chunks
NKC = (S) // 128     # 8 key chunks per batch
NFFC = DFF // 128    # 32
VSH = V // NC        # 4000 vocab per core
VPAD = 4096          # padded vocab slice
EPS = 1e-5
HKV = TOK * D        # elements per kv AllGather half (K half + V half)


def _pe():
    pos = np.arange(S, dtype=np.float32)[:, None]
    div = np.exp(np.arange(0, D, 2, dtype=np.float32) * (-np.log(10000.0) / D))
    pe = np.zeros((S, D), dtype=np.float32)
    pe[:, 0::2] = np.sin(pos * div)
    pe[:, 1::2] = np.cos(pos * div)
    return pe


def build_program(ln1_triv, ln2_triv, b2_triv, lnf_triv, pb_triv):
    nc = bacc.Bacc("TRN2", target_bir_lowering=False, debug=False,
                   enable_asserts=False, num_devices=NC)

    # ---- DRAM inputs (shared weights) ----
    wq_d = nc.dram_tensor("wq", [L, D, D], bf16, kind="ExternalInput").ap()
    wk_d = nc.dram_tensor("wk", [L, D, D], bf16, kind="ExternalInput").ap()
    wv_d = nc.dram_tensor("wv", [L, D, D], bf16, kind="ExternalInput").ap()
    wo_d = nc.dram_tensor("wo", [L, D, D], bf16, kind="ExternalInput").ap()
    w1_d = nc.dram_tensor("w1", [L, D, DFF], bf16, kind="ExternalInput").ap()
    w2_d = nc.dram_tensor("w2", [L, DFF, D], bf16, kind="ExternalInput").ap()
    b1_d = nc.dram_tensor("b1", [L, DFF], f32, kind="ExternalInput").ap()
    # ---- per-core inputs ----
    embx_d = nc.dram_tensor("embx", [TOK, D], f32, kind="ExternalInput").ap()
    pe_d = nc.dram_tensor("pe", [TOK, D], f32, kind="ExternalInput").ap()
    mt_d = nc.dram_tensor("mt", [NKC, 128, TOK], bf16, kind="ExternalInput").ap()
    pw_d = nc.dram_tensor("pw", [D, VPAD], bf16, kind="ExternalInput").ap()
    if not pb_triv:
        pbb_d = nc.dram_tensor("pbb", [128, VPAD], f32, kind="ExternalInput").ap()
    # optional non-trivial affine params (pre-broadcast on host)
    if not (ln1_triv and ln2_triv):
        lngb_d = nc.dram_tensor("lngb", [L, 4, 128, D], f32, kind="ExternalInput").ap()
    if not b2_triv:
        b2b_d = nc.dram_tensor("b2b", [L, 128, D], f32, kind="ExternalInput").ap()
    if not lnf_triv:
        fgb_d = nc.dram_tensor("fgb", [2, 128, D], f32, kind="ExternalInput").ap()
    # ---- output: token-major logits ----
    out_d = nc.dram_tensor("logits", [B * S, VPAD], f32, kind="ExternalOutput").ap()

    from contextlib import ExitStack
    with tile.TileContext(nc) as tc:
        with ExitStack() as ctx0:
            # pools that live for the whole program
            cpool = ctx0.enter_context(tc.tile_pool(name="const", bufs=1))
            dpool = ctx0.enter_context(tc.tile_pool(name="dram", bufs=1, space="DRAM"))

            # ---------------- constants ----------------
            ident = cpool.tile([128, 128], bf16, tag="ident")
            make_identity(nc, ident[:])
            ones = cpool.tile([128, 64], bf16, tag="ones")
            nc.vector.memset(ones[:], 1.0)
            epst = cpool.tile([128, 1], f32, tag="eps")
            nc.vector.memset(epst[:], EPS)
            mtt = cpool.tile([128, NKC, TOK], bf16, tag="mt")
            nc.sync.dma_start(out=mtt[:], in_=mt_d.rearrange("kc p q -> p kc q"))
            b1t = cpool.tile([128, L, NFFC], f32, tag="b1")
            nc.sync.dma_start(out=b1t[:], in_=b1_d.rearrange("l (fc p) -> p l fc", p=128))
            if not (ln1_triv and ln2_triv):
                lngb = cpool.tile([128, L, 4, D], f32, tag="lngb")
                nc.sync.dma_start(out=lngb[:], in_=lngb_d.rearrange("l f p d -> p l f d"))
            if not b2_triv:
                b2b = cpool.tile([128, L, D], f32, tag="b2b")
                nc.sync.dma_start(out=b2b[:], in_=b2b_d.rearrange("l p d -> p l d"))
            if not lnf_triv:
                fgb = cpool.tile([128, 2, D], f32, tag="fgb")
                nc.sync.dma_start(out=fgb[:], in_=fgb_d.rearrange("f p d -> p f d"))

            # collective DRAM buffers: kv AllGather in 2 head-halves
            kv_in = [dpool.tile([HKV], bf16, tag=f"kv_in{g}", name=f"kv_in{g}")
                     for g in range(2)]
            kv_out = [dpool.tile([4, HKV], bf16, tag=f"kv_out{g}", name=f"kv_out{g}")
                      for g in range(2)]
            ag2_in = dpool.tile([TOK * D], bf16, tag="ag2_in")
            ag2_out = dpool.tile([NC, TOK * D], bf16, tag="ag2_out", addr_space="Shared")

            with ExitStack() as ctx:
                rpool = ctx.enter_context(tc.tile_pool(name="resid", bufs=1))
                atpool = ctx.enter_context(tc.tile_pool(name="aT", bufs=2))
                qkvpool = ctx.enter_context(tc.tile_pool(name="qkv", bufs=1))
                vapool = ctx.enter_context(tc.tile_pool(name="vall", bufs=1))
                wpool = ctx.enter_context(tc.tile_pool(name="wch", bufs=4))
                w2pool = ctx.enter_context(tc.tile_pool(name="w2c", bufs=3))
                utpool = ctx.enter_context(tc.tile_pool(name="ut", bufs=32))
                htpool = ctx.enter_context(tc.tile_pool(name="ht", bufs=4))
                apool = ctx.enter_context(tc.tile_pool(name="att", bufs=8))
                alnpool = ctx.enter_context(tc.tile_pool(name="aln", bufs=2))
                spool = ctx.enter_context(tc.tile_pool(name="small", bufs=4))
                recpool = ctx.enter_context(tc.tile_pool(name="recp", bufs=2))
                psmm = ctx.enter_context(tc.tile_pool(name="ps_mm", bufs=4, space="PSUM"))
                pso = ctx.enter_context(tc.tile_pool(name="ps_o", bufs=4, space="PSUM"))

                # resident V (+ones col) for attention: [k, kc, head, dh|1]
                vall = vapool.tile([128, NKC, H, DH + 1], bf16, tag="vall", name="vall")
                nc.vector.memset(vall[:], 1.0)

                # ---------------- embedding ----------------
                h = [rpool.tile([128, D], f32, tag=f"h{tb}", name=f"h{tb}") for tb in range(NTB)]
                for tb in range(NTB):
                    et = alnpool.tile([128, D], f32, tag="aln")
                    pt = alnpool.tile([128, D], f32, tag="aln")
                    nc.sync.dma_start(out=et, in_=embx_d[tb * 128:(tb + 1) * 128, :])
                    nc.sync.dma_start(out=pt, in_=pe_d[tb * 128:(tb + 1) * 128, :])
                    nc.vector.tensor_scalar(out=et[:], in0=et[:], scalar1=float(np.sqrt(D)),
                                            scalar2=None, op0=mybir.AluOpType.mult)
                    nc.vector.tensor_add(h[tb][:], et[:], pt[:])

                def layer_norm(src_tiles, dst_tag, gb=None):
                    """LN along free dim of token-major tiles; returns new tiles."""
                    outs = []
                    for tb in range(NTB):
                        st = spool.tile([128, 2, 6], f32, tag="bnst")
                        xin = src_tiles[tb][:].rearrange("p (g d) -> p g d", g=2)
                        for g in range(2):
                            nc.vector.bn_stats(out=st[:, g, :], in_=xin[:, g, :])
                        mv = spool.tile([128, 2], f32, tag="bnmv")
                        nc.vector.bn_aggr(out=mv[:], in_=st[:])
                        std = spool.tile([128, 1], f32, tag="bnsd")
                        nc.scalar.activation(out=std[:], in_=mv[:, 1:2],
                                             func=mybir.ActivationFunctionType.Sqrt,
                                             bias=epst[:], scale=1.0)
                        nc.vector.reciprocal(out=std[:], in_=std[:])
                        at = alnpool.tile([128, D], bf16, tag=dst_tag)
                        nc.vector.tensor_scalar(out=at[:], in0=src_tiles[tb][:],
                                                scalar1=mv[:, 0:1], scalar2=std[:],
                                                op0=mybir.AluOpType.subtract,
                                                op1=mybir.AluOpType.mult)
                        if gb is not None:
                            gt, bt = gb
                            nc.vector.tensor_mul(at[:], at[:], gt)
                            nc.vector.tensor_add(at[:], at[:], bt)
                        outs.append(at)
                    return outs

                def transpose_to_aT(src_tiles):
                    """token-major [128, D] x NTB -> aT [128, NDC, TOK]."""
                    at = atpool.tile([128, NDC, TOK], bf16, tag="aT")
                    for tb in range(NTB):
                        for dc in range(NDC):
                            ps = psmm.tile([128, 128], bf16, tag="mm")
                            nc.tensor.transpose(ps[:], src_tiles[tb][:, dc * 128:(dc + 1) * 128], ident[:])
                            nc.vector.tensor_copy(out=at[:, dc, tb * 128:(tb + 1) * 128], in_=ps[:])
                    return at

                # ---------------- layers ----------------
                for li in range(L):
                    ln1_gb = None
                    if not ln1_triv:
                        ln1_gb = (lngb[:, li, 0, :], lngb[:, li, 1, :])
                    aln = layer_norm(h, "aln", ln1_gb)
                    aT = transpose_to_aT(aln)

                    # --- K/V projections + AllGather, split in two head-halves ---
                    kloc = qkvpool.tile([128, NDC, TOK], bf16, tag="kloc")
                    vloc = [qkvpool.tile([128, D], bf16, tag=f"vloc{tb}", name=f"vloc{tb}")
                            for tb in range(NTB)]
                    wk_src = wk_d[li].rearrange("(dc p) j -> p dc j", p=128)
                    wv_src = wv_d[li].rearrange("(dc p) j -> p dc j", p=128)
                    for g in range(2):
                        # K proj: head-half g => j-chunks 4g..4g+3
                        wc = wpool.tile([128, NDC, 512], bf16, tag="wch")
                        nc.sync.dma_start(out=wc, in_=wk_src[:, :, g * 512:(g + 1) * 512])
                        for oc in range(4):
                            jc = g * 4 + oc
                            ps = psmm.tile([128, TOK], f32, tag="mm")
                            for dc in range(NDC):
                                nc.tensor.matmul(ps[:], wc[:, dc, oc * 128:(oc + 1) * 128],
                                                 aT[:, dc, :], start=(dc == 0), stop=(dc == NDC - 1))
                            nc.vector.tensor_copy(out=kloc[:, jc, :], in_=ps[:])
                        # V proj: dims g*512..g*512+512 (token-major out)
                        wc = wpool.tile([128, NDC, 512], bf16, tag="wch")
                        nc.sync.dma_start(out=wc, in_=wv_src[:, :, g * 512:(g + 1) * 512])
                        for tb in range(NTB):
                            ps = psmm.tile([128, 512], f32, tag="mm")
                            for dc in range(NDC):
                                nc.tensor.matmul(ps[:], aT[:, dc, tb * 128:(tb + 1) * 128],
                                                 wc[:, dc, :], start=(dc == 0), stop=(dc == NDC - 1))
                            nc.vector.tensor_copy(out=vloc[tb][:, g * 512:(g + 1) * 512], in_=ps[:])
                        # pack + AllGather this half
                        nc.sync.dma_start(
                            out=kv_in[g][0:HKV // 2].rearrange("(jc p k) -> p jc k", p=128, k=TOK),
                            in_=kloc[:, g * 4:(g + 1) * 4, :])
                        for tb in range(NTB):
                            nc.sync.dma_start(
                                out=kv_in[g][HKV // 2:HKV].rearrange(
                                    "(tb p j) -> p tb j", p=128, j=512)[:, tb, :],
                                in_=vloc[tb][:, g * 512:(g + 1) * 512])
                        nc.gpsimd.collective_compute(
                            "AllGather", mybir.AluOpType.bypass,
                            replica_groups=[[0, 1, 2, 3], [4, 5, 6, 7]],
                            ins=[kv_in[g][:].opt()], outs=[kv_out[g][:].opt()])

                    # --- Q projection (overlaps the AllGathers) ---
                    qT = qkvpool.tile([128, NDC, TOK], bf16, tag="qT")
                    wsrc = wq_d[li].rearrange("(dc p) j -> p dc j", p=128)
                    for cg in range(2):
                        wc = wpool.tile([128, NDC, 512], bf16, tag="wch")
                        nc.sync.dma_start(out=wc, in_=wsrc[:, :, cg * 512:(cg + 1) * 512])
                        for oc in range(4):
                            jc = cg * 4 + oc
                            ps = psmm.tile([128, TOK], f32, tag="mm")
                            for dc in range(NDC):
                                nc.tensor.matmul(ps[:], wc[:, dc, oc * 128:(oc + 1) * 128],
                                                 aT[:, dc, :], start=(dc == 0), stop=(dc == NDC - 1))
                            nc.vector.tensor_copy(out=qT[:, jc, :], in_=ps[:])

                    # --- attention ---
                    oall = qkvpool.tile([128, NDC, TOK], bf16, tag="oall")
                    for g in range(2):
                        kv_k = kv_out[g][:, 0:HKV // 2].rearrange(
                            "s (jc p k) -> p s jc k", jc=4, p=128, k=TOK)
                        kv_v = kv_out[g][:, HKV // 2:HKV].rearrange(
                            "s (tb p j) -> p s tb j", tb=NTB, p=128, j=512)
                        # K tile for the first head-pair goes first: scores can
                        # start the moment the gather lands (AV needs vall later).
                        ktiles = [htpool.tile([128, 4, TOK], bf16, tag="kt", name=f"kt{g}{hl}")
                                  for hl in range(4)]
                        nc.scalar.dma_start(out=ktiles[0], in_=kv_k[:, :, 0, :])
                        # V (+ones) into resident vall for heads of this half
                        for s in range(4):
                            for tb in range(NTB):
                                nc.scalar.dma_start(
                                    out=vall[:, s * 2 + tb, g * 8:(g + 1) * 8, 0:DH],
                                    in_=kv_v[:, s, tb, :].rearrange("p (hh d) -> p hh d", d=DH))
                        for hl in range(1, 4):
                            nc.scalar.dma_start(out=ktiles[hl], in_=kv_k[:, :, hl, :])
                        for hl in range(4):
                            hp = g * 4 + hl
                            ktile = ktiles[hl]
                            ps_o = [pso.tile([128, TOK], f32, tag="o", name=f"pso{hh}")
                                    for hh in range(2)]
                            at2s = [None, None]
                            for kg in range(4):
                                prev = at2s
                                at2s = [None, None]
                                for hh in range(2):
                                    hb = hh * 64
                                    ps_s = psmm.tile([128, 512], f32, tag="mm")
                                    for k2 in range(2):
                                        kc = kg * 2 + k2
                                        nc.tensor.matmul(
                                            ps_s[:, k2 * 256:(k2 + 1) * 256],
                                            ktile[hb:hb + 64, kc // 2, (kc % 2) * 128:(kc % 2) * 128 + 128],
                                            qT[hb:hb + 64, hp, :], start=True, stop=True)
                                    et = apool.tile([128, 512], bf16, tag="att")
                                    nc.scalar.activation(out=et[:], in_=ps_s[:],
                                                         func=mybir.ActivationFunctionType.Exp,
                                                         scale=float(1.0 / np.sqrt(DH)))
                                    at2 = apool.tile([128, 512], bf16, tag="att")
                                    nc.gpsimd.tensor_mul(
                                        at2[:], et[:],
                                        mtt[:, kg * 2:(kg + 1) * 2, :].rearrange("p a b -> p (a b)"))
                                    at2s[hh] = at2
                                if kg > 0:
                                    for hh in range(2):
                                        for k2 in range(2):
                                            kc = (kg - 1) * 2 + k2
                                            nc.tensor.matmul(
                                                ps_o[hh][0:DH + 1, :],
                                                vall[:, kc, hp * 2 + hh, :],
                                                prev[hh][:, k2 * 256:(k2 + 1) * 256],
                                                start=(kc == 0), stop=False)
                            for hh in range(2):
                                for k2 in range(2):
                                    kc = 6 + k2
                                    nc.tensor.matmul(
                                        ps_o[hh][0:DH + 1, :],
                                        vall[:, kc, hp * 2 + hh, :],
                                        at2s[hh][:, k2 * 256:(k2 + 1) * 256],
                                        start=False, stop=(kc == NKC - 1))
                            for hh in range(2):
                                hb = hh * 64
                                rec = recpool.tile([128, TOK], bf16, tag="rec")
                                with nc.allow_low_precision(reason="softmax denom recip"):
                                    nc.vector.reciprocal(out=rec[DH:DH + 1, :],
                                                         in_=ps_o[hh][DH:DH + 1, :])
                                ps_rb = pso.tile([128, TOK], f32, tag="o", name=f"psrb{hh}")
                                nc.tensor.matmul(ps_rb[0:64, :], ones[DH:DH + 1, 0:64],
                                                 rec[DH:DH + 1, :], start=True, stop=True)
                                rb = apool.tile([128, TOK], f32, tag="attf")
                                nc.vector.tensor_copy(out=rb[0:64, :], in_=ps_rb[0:64, :])
                                nc.vector.tensor_mul(oall[hb:hb + 64, hp, :],
                                                     ps_o[hh][0:64, :], rb[0:64, :])

                    # --- Wo projection (token-major out) + residual ---
                    wsrc = wo_d[li].rearrange("(jc p) o -> p jc o", p=128)
                    for cg in range(2):
                        wc = wpool.tile([128, NDC, 512], bf16, tag="wch")
                        nc.sync.dma_start(out=wc, in_=wsrc[:, :, cg * 512:(cg + 1) * 512])
                        for tb in range(NTB):
                            ps = psmm.tile([128, 512], f32, tag="mm")
                            for jc in range(NDC):
                                nc.tensor.matmul(ps[:], oall[:, jc, tb * 128:(tb + 1) * 128],
                                                 wc[:, jc, :], start=(jc == 0), stop=(jc == NDC - 1))
                            nc.vector.tensor_add(h[tb][:, cg * 512:(cg + 1) * 512],
                                                 h[tb][:, cg * 512:(cg + 1) * 512], ps[:])

                    # --- FFN ---
                    ln2_gb = None
                    if not ln2_triv:
                        ln2_gb = (lngb[:, li, 2, :], lngb[:, li, 3, :])
                    f_ln = layer_norm(h, "aln", ln2_gb)
                    fT = transpose_to_aT(f_ln)

                    uts = []
                    wsrc = w1_d[li].rearrange("(dc p) j -> p dc j", p=128)
                    for cg in range(NFFC // 4):
                        wc = wpool.tile([128, NDC, 512], bf16, tag="wch")
                        nc.sync.dma_start(out=wc, in_=wsrc[:, :, cg * 512:(cg + 1) * 512])
                        for oc in range(4):
                            fc = cg * 4 + oc
                            ps = psmm.tile([128, TOK], f32, tag="mm")
                            for dc in range(NDC):
                                nc.tensor.matmul(ps[:], wc[:, dc, oc * 128:(oc + 1) * 128],
                                                 fT[:, dc, :], start=(dc == 0), stop=(dc == NDC - 1))
                            ut = utpool.tile([128, TOK], bf16, tag="ut")
                            nc.vector.tensor_scalar(out=ut[:], in0=ps[:],
                                                    scalar1=b1t[:, li, fc:fc + 1], scalar2=0.0,
                                                    op0=mybir.AluOpType.add,
                                                    op1=mybir.AluOpType.max)
                            uts.append(ut)

                    # --- W2: 4 psum chains (tb x og), accumulate over ffc ---
                    wsrc = w2_d[li].rearrange("(fc p) o -> p fc o", p=128)
                    chains = {}
                    for tb in range(NTB):
                        for og in range(2):
                            chains[(tb, og)] = psmm.tile([128, 512], f32, tag="mm", name=f"w2ch{tb}{og}")
                    for fcg in range(NFFC // 2):
                        wc = w2pool.tile([128, 2, D], bf16, tag="w2c")
                        nc.sync.dma_start(out=wc, in_=wsrc[:, fcg * 2:fcg * 2 + 2, :])
                        for f2 in range(2):
                            fc = fcg * 2 + f2
                            for tb in range(NTB):
                                for og in range(2):
                                    nc.tensor.matmul(chains[(tb, og)][:],
                                                     uts[fc][:, tb * 128:(tb + 1) * 128],
                                                     wc[:, f2, og * 512:(og + 1) * 512],
                                                     start=(fc == 0), stop=(fc == NFFC - 1))
                    for tb in range(NTB):
                        for og in range(2):
                            nc.vector.tensor_add(h[tb][:, og * 512:(og + 1) * 512],
                                                 h[tb][:, og * 512:(og + 1) * 512],
                                                 chains[(tb, og)][:])
                        if not b2_triv:
                            nc.vector.tensor_add(h[tb][:], h[tb][:], b2b[:, li, :])

                # ---------------- final LN + AllGather ----------------
                fin_gb = None if lnf_triv else (fgb[:, 0, :], fgb[:, 1, :])
                fin = layer_norm(h, "aln", fin_gb)
                finT = transpose_to_aT(fin)
                nc.sync.dma_start(
                    out=ag2_in[:].rearrange("(jc p t) -> p jc t", p=128, t=TOK),
                    in_=finT[:])
                nc.gpsimd.collective_compute(
                    "AllGather", mybir.AluOpType.bypass,
                    replica_groups=[[0, 1, 2, 3, 4, 5, 6, 7]],
                    ins=[ag2_in[:].opt()], outs=[ag2_out[:].opt()])

            # ---------------- vocab projection (fresh pools) ----------------
            with ExitStack() as ctx2:
                ppool = ctx2.enter_context(tc.tile_pool(name="pw", bufs=2))
                hpool2 = ctx2.enter_context(tc.tile_pool(name="hts", bufs=3))
                epool = ctx2.enter_context(tc.tile_pool(name="evac", bufs=4))
                pchain = ctx2.enter_context(tc.tile_pool(name="ps_p", bufs=4, space="PSUM"))

                if not pb_triv:
                    pbbt = cpool.tile([128, VPAD], f32, tag="pbb")
                    nc.sync.dma_start(out=pbbt[:], in_=pbb_d)

                hfull = ag2_out[:].rearrange("s (jc p t) -> p s jc t", jc=NDC, p=128, t=TOK)
                pwsrc = pw_d.rearrange("(dc p) v -> p dc v", p=128)
                for vh in range(2):
                    pwall = ppool.tile([128, NDC, 2048], bf16, tag="pwall")
                    nc.sync.dma_start(out=pwall, in_=pwsrc[:, :, vh * 2048:(vh + 1) * 2048])
                    for ts in range(16):
                        sr, tb2 = ts // 2, ts % 2
                        hts = hpool2.tile([128, NDC, 128], bf16, tag="hts")
                        nc.sync.dma_start(out=hts, in_=hfull[:, sr, :, tb2 * 128:(tb2 + 1) * 128])
                        for vq in range(4):
                            ps = pchain.tile([128, 512], f32, tag="pj")
                            for dc in range(NDC):
                                nc.tensor.matmul(ps[:], hts[:, dc, :],
                                                 pwall[:, dc, vq * 512:(vq + 1) * 512],
                                                 start=(dc == 0), stop=(dc == NDC - 1))
                            lsb = epool.tile([128, 512], f32, tag="ev")
                            if pb_triv:
                                nc.vector.tensor_copy(out=lsb[:], in_=ps[:])
                            else:
                                nc.vector.tensor_add(
                                    lsb[:], ps[:],
                                    pbbt[:, vh * 2048 + vq * 512: vh * 2048 + (vq + 1) * 512])
                            nc.sync.dma_start(
                                out=out_d[ts * 128:(ts + 1) * 128,
                                          vh * 2048 + vq * 512: vh * 2048 + (vq + 1) * 512],
                                in_=lsb[:])
    nc.compile()
    return nc


def kernel(**inputs):
    x = np.asarray(inputs["x"])
    mask = np.asarray(inputs["mask"])
    emb = np.asarray(inputs["emb"], dtype=np.float32)

    ln1_g = np.asarray(inputs["ln1_g"], dtype=np.float32)
    ln1_b = np.asarray(inputs["ln1_b"], dtype=np.float32)
    ln2_g = np.asarray(inputs["ln2_g"], dtype=np.float32)
    ln2_b = np.asarray(inputs["ln2_b"], dtype=np.float32)
    lnf_g = np.asarray(inputs["lnf_g"], dtype=np.float32)
    lnf_b = np.asarray(inputs["lnf_b"], dtype=np.float32)
    b2 = np.asarray(inputs["b2"], dtype=np.float32)
    pb_full = np.asarray(inputs["projb"], dtype=np.float32)

    ln1_triv = bool(np.all(ln1_g == 1) and np.all(ln1_b == 0))
    ln2_triv = bool(np.all(ln2_g == 1) and np.all(ln2_b == 0))
    lnf_triv = bool(np.all(lnf_g == 1) and np.all(lnf_b == 0))
    b2_triv = bool(np.all(b2 == 0))
    pb_triv = bool(np.all(pb_full == 0))

    nc = build_program(ln1_triv, ln2_triv, b2_triv, lnf_triv, pb_triv)

    pe_full = _pe()
    ids = np.asarray(x).reshape(B * S)
    m2d = np.asarray(mask[0, 0], dtype=np.float32)  # [S(q), S(k)]
    pw_full = np.asarray(inputs["projW"], dtype=np.float32)

    import ml_dtypes
    bfl = ml_dtypes.bfloat16
    shared = {
        "wq": np.ascontiguousarray(np.asarray(inputs["Wq"], dtype=bfl)),
        "wk": np.ascontiguousarray(np.asarray(inputs["Wk"], dtype=bfl)),
        "wv": np.ascontiguousarray(np.asarray(inputs["Wv"], dtype=bfl)),
        "wo": np.ascontiguousarray(np.asarray(inputs["Wo"], dtype=bfl)),
        "w1": np.ascontiguousarray(np.asarray(inputs["W1"], dtype=bfl)),
        "w2": np.ascontiguousarray(np.asarray(inputs["W2"], dtype=bfl)),
        "b1": np.ascontiguousarray(inputs["b1"], dtype=np.float32),
    }
    if not (ln1_triv and ln2_triv):
        lngb = np.stack([
            np.broadcast_to(ln1_g[:, None, :], (L, 128, D)),
            np.broadcast_to(ln1_b[:, None, :], (L, 128, D)),
            np.broadcast_to(ln2_g[:, None, :], (L, 128, D)),
            np.broadcast_to(ln2_b[:, None, :], (L, 128, D)),
        ], axis=1)
        shared["lngb"] = np.ascontiguousarray(lngb, dtype=np.float32)
    if not b2_triv:
        shared["b2b"] = np.ascontiguousarray(
            np.broadcast_to(b2[:, None, :], (L, 128, D)), dtype=np.float32)
    if not lnf_triv:
        shared["fgb"] = np.ascontiguousarray(
            np.stack([np.broadcast_to(lnf_g[None, :], (128, D)),
                      np.broadcast_to(lnf_b[None, :], (128, D))]), dtype=np.float32)

    in_maps = []
    for c in range(NC):
        b = c // 4
        q0 = (c % 4) * TOK
        sl = slice(b * S + q0, b * S + q0 + TOK)
        embx = np.ascontiguousarray(emb[ids[sl]], dtype=np.float32)
        pes = np.ascontiguousarray(pe_full[q0:q0 + TOK], dtype=np.float32)
        # mask tiles in scores-T layout: mt[kc, k, q] = mask[q0+q, kc*128+k]
        msl = m2d[q0:q0 + TOK, :]  # [TOK, S]
        mt = np.ascontiguousarray(
            np.asarray(msl.T.reshape(NKC, 128, TOK), dtype=bfl))
        pw = np.zeros((D, VPAD), dtype=bfl)
        pw[:, :VSH] = np.asarray(pw_full[:, c * VSH:(c + 1) * VSH], dtype=bfl)
        im = dict(shared)
        im.update({"embx": embx, "pe": pes, "mt": mt, "pw": pw})
        if not pb_triv:
            pbb = np.zeros((128, VPAD), dtype=np.float32)
            pbb[:, :VSH] = pb_full[None, c * VSH:(c + 1) * VSH]
            im["pbb"] = np.ascontiguousarray(pbb)
        in_maps.append(im)

    trace = bool(int(os.environ.get("KERNEL_TRACE", "0")))
    if trace:
        _install_trace_hook()
    res = run_bass_kernel_spmd(nc, in_maps, core_ids=list(range(NC)), trace=trace)
    if trace:
        kernel.last_exec_time_ns = res.exec_time_ns

    parts = [res.results[c]["logits"][:, :VSH] for c in range(NC)]
    full = np.concatenate(parts, axis=1)          # [B*S, V]
    return np.ascontiguousarray(full.reshape(B, S, V))


def _install_trace_hook():
    import types
    if 'antenv.axon_hooks' in sys.modules:
        return
    try:
        import trn_agent_boot.trn_boot as trn_boot
        mod = types.ModuleType('antenv.axon_hooks')
        _hook = [None]
        mod.set_axon_ntff_profile_hook = lambda hk: _hook.__setitem__(0, hk)
        mod.get_axon_ntff_profile_hook = lambda: _hook[0]
        sys.modules['antenv.axon_hooks'] = mod
        import antenv
        antenv.axon_hooks = mod
        mod.set_axon_ntff_profile_hook(
            trn_boot._ntff_profile_via_ctypes('/opt/axon/libaxon_pjrt.so'))
    except Exception as e:
        print(f"trace hook unavailable: {e}", file=sys.stderr)
